# revision 11
# baseline (speedup 1.0000x reference)
"""Trainium2 Bass kernel for one GPT-style transformer block.

Problem: x[8,1024,1024]; per-core = one batch element (data-parallel over 8
NeuronCores).  Per core:
    h1 = LN(x); qkv = h1@Wqkv+b; causal MHA (16 heads, d=64);
    r1 = x + attn@Wproj+b; h2 = LN(r1); out = r1 + relu(h2@W1+b1)@W2+b2

Design notes:
  - Activations live feature-major in SBUF: [C partition, T free], so every
    linear layer is matmul(lhsT=W[K=Cin,M=Cout], rhs=act[K=Cin,N=T]) with no
    activation transposes.
  - Matmuls run as float32r (TF32-like, full PE rate at N>=256).  The BIR
    verifier requires every f32r-matmul operand to be *produced* as f32r, so
    matmul-feeding tiles are typed float32r (engines round on write; DMA from
    f32r-typed DRAM passes through).  memset cannot write f32r -> constants
    are memset f32 then ACT-copied.
  - Attention computes S^T = (K-block)^T @ Q chunks -> [Tk,Tq] tiles; softmax
    denominators come for free by augmenting V^T with a ones column in the
    P@V matmul (row 64 of the PV psum = sum_k exp).  Causal masking via
    gpsimd.affine_select on the exp'd tiles.  No max-subtraction needed:
    |S| <= ~10 so exp never overflows fp32.
  - LN stats (sums over the partition axis) via ones-vector matmuls; the
    per-token scale/shift rows are broadcast across partitions with a rank-1
    (ones outer row) matmul.
  - FFN runs in two d_ff halves so the fp32 intermediate fits SBUF.
"""

import math
import sys

import numpy as np

sys.path.insert(0, "/opt/trn_rl_repo")

from contextlib import ExitStack

import concourse.bass as bass
import concourse.mybir as mybir
import concourse.tile as tile
from concourse import bacc
from concourse.bass import ts
from concourse.masks import make_identity

F32 = mybir.dt.float32
F32R = mybir.dt.float32r
AF = mybir.ActivationFunctionType

B, T, C, H = 8, 1024, 1024, 16
D = C // H
FF = 4 * C
P = 128
NCH = C // P          # 8 feature chunks
NT = T // P           # 8 token chunks of 128
NQ = T // 512         # 2 token chunks of 512
SCALE = 1.0 / math.sqrt(3 * C // H)
EPS = 1e-5


def _build():
    nc = bacc.Bacc("TRN2", target_bir_lowering=False, debug=False)

    x_d = nc.dram_tensor("x", [T, C], F32, kind="ExternalInput").ap()
    Wqkv_d = nc.dram_tensor("Wqkv", [C, 3 * C], F32R, kind="ExternalInput").ap()
    bqkv_d = nc.dram_tensor("bqkv", [3 * C], F32, kind="ExternalInput").ap()
    Wproj_d = nc.dram_tensor("Wproj", [C, C], F32R, kind="ExternalInput").ap()
    bproj_d = nc.dram_tensor("bproj", [C], F32, kind="ExternalInput").ap()
    ln1g_d = nc.dram_tensor("ln1_g", [C], F32, kind="ExternalInput").ap()
    ln1b_d = nc.dram_tensor("ln1_b", [C], F32, kind="ExternalInput").ap()
    ln2g_d = nc.dram_tensor("ln2_g", [C], F32, kind="ExternalInput").ap()
    ln2b_d = nc.dram_tensor("ln2_b", [C], F32, kind="ExternalInput").ap()
    W1_d = nc.dram_tensor("W1", [C, FF], F32R, kind="ExternalInput").ap()
    b1_d = nc.dram_tensor("b1", [FF], F32, kind="ExternalInput").ap()
    W2_d = nc.dram_tensor("W2", [FF, C], F32R, kind="ExternalInput").ap()
    b2_d = nc.dram_tensor("b2", [C], F32, kind="ExternalInput").ap()
    out_d = nc.dram_tensor("out", [T, C], F32, kind="ExternalOutput").ap()

    Wqkv_r = Wqkv_d.rearrange("(j p) m -> p j m", p=P)     # [128, 8, 3072]
    Wproj_r = Wproj_d.rearrange("(j p) m -> p j m", p=P)   # [128, 8, 1024]
    W1_r = W1_d.rearrange("(j p) m -> p j m", p=P)         # [128, 8, 4096]
    W2_r = W2_d.rearrange("(j p) m -> p j m", p=P)         # [128, 32, 1024]

    with nc.allow_low_precision(reason="fp32r matmul inputs (fp32 accum)"), \
         tile.TileContext(nc) as tc, ExitStack() as ctx:
        const = ctx.enter_context(tc.tile_pool(name="const", bufs=1))
        xpool = ctx.enter_context(tc.tile_pool(name="xpool", bufs=8))
        hpool = ctx.enter_context(tc.tile_pool(name="hpool", bufs=8))
        spool = ctx.enter_context(tc.tile_pool(name="spool", bufs=2))
        wpool = ctx.enter_context(tc.tile_pool(name="wpool", bufs=2))
        ps_mm = ctx.enter_context(tc.tile_pool(name="ps_mm", bufs=4, space="PSUM"))
        ps_pv = ctx.enter_context(tc.tile_pool(name="ps_pv", bufs=2, space="PSUM"))
        ps_tr = ctx.enter_context(tc.tile_pool(name="ps_tr", bufs=2, space="PSUM"))

        ident = const.tile([P, P], F32)
        make_identity(nc, ident[:])
        ident_r = const.tile([P, P], F32R)
        nc.scalar.activation(ident_r[:], ident[:], AF.Copy)
        ones_f = const.tile([P, 1], F32)
        nc.vector.memset(ones_f[:], 1.0)
        ones_col = const.tile([P, 1], F32R)
        nc.scalar.activation(ones_col[:], ones_f[:], AF.Copy)
        ones_rowf = const.tile([1, P], F32)
        nc.vector.memset(ones_rowf[:], 1.0)
        ones_row = const.tile([1, P], F32R)
        nc.scalar.activation(ones_row[:], ones_rowf[:], AF.Copy)
        eps_t = const.tile([1, 1], F32)
        nc.vector.memset(eps_t[:], EPS)
        zero_col = const.tile([P, 1], F32)
        nc.vector.memset(zero_col[:], 0.0)

        # bias/param columns: col m = vec[m*128:(m+1)*128]
        bqkv_t = const.tile([P, 3 * NCH], F32)
        nc.sync.dma_start(bqkv_t[:], bqkv_d.rearrange("(m p) -> p m", p=P))
        bproj_t = const.tile([P, NCH], F32)
        nc.sync.dma_start(bproj_t[:], bproj_d.rearrange("(m p) -> p m", p=P))
        b1_t = const.tile([P, FF // P], F32)
        nc.sync.dma_start(b1_t[:], b1_d.rearrange("(m p) -> p m", p=P))
        b2_t = const.tile([P, NCH], F32)
        nc.sync.dma_start(b2_t[:], b2_d.rearrange("(m p) -> p m", p=P))
        ln1g_t = const.tile([P, NCH], F32)
        nc.sync.dma_start(ln1g_t[:], ln1g_d.rearrange("(m p) -> p m", p=P))
        ln1b_t = const.tile([P, NCH], F32)
        nc.sync.dma_start(ln1b_t[:], ln1b_d.rearrange("(m p) -> p m", p=P))
        ln2g_t = const.tile([P, NCH], F32)
        nc.sync.dma_start(ln2g_t[:], ln2g_d.rearrange("(m p) -> p m", p=P))
        ln2b_t = const.tile([P, NCH], F32)
        nc.sync.dma_start(ln2b_t[:], ln2b_d.rearrange("(m p) -> p m", p=P))

        # persistent feature-major x tiles; become r1 then out in place
        x_t = [xpool.tile([P, T], F32R, tag="x", name=f"x_fm{m}") for m in range(NCH)]

        def layernorm_fm(src, g_t, b_t, out_tag, out_name):
            """src: 8 [128,1024] FM f32r tiles -> 8 normalized FM f32r tiles."""
            sum_ps = [ps_mm.tile([1, 512], F32, tag="ps", name=f"{out_name}_sum{t}")
                      for t in range(NQ)]
            sq_ps = [ps_mm.tile([1, 512], F32, tag="ps", name=f"{out_name}_sq{t}")
                     for t in range(NQ)]
            for c in range(NCH):
                for t in range(NQ):
                    sq = spool.tile([P, 512], F32R, tag="sq",
                                    name=f"{out_name}_sqv{c}_{t}")
                    nc.vector.tensor_mul(sq[:], src[c][:, ts(t, 512)],
                                         src[c][:, ts(t, 512)])
                    nc.tensor.matmul(
                        sum_ps[t][:], ones_col[:], src[c][:, ts(t, 512)],
                        start=(c == 0), stop=(c == NCH - 1))
                    nc.tensor.matmul(
                        sq_ps[t][:], ones_col[:], sq[:],
                        start=(c == 0), stop=(c == NCH - 1))
            inv_t = spool.tile([1, T], F32R, tag="lnstat", name=f"{out_name}_inv")
            c0_t = spool.tile([1, T], F32R, tag="lnstat", name=f"{out_name}_c0")
            for t in range(NQ):
                mu = spool.tile([1, 512], F32R, tag="sm512", bufs=6,
                                name=f"{out_name}_mu{t}")
                var = spool.tile([1, 512], F32, tag="sm512", bufs=6,
                                 name=f"{out_name}_var{t}")
                nc.scalar.mul(mu[:], sum_ps[t][:], 1.0 / C)
                nc.scalar.mul(var[:], sq_ps[t][:], 1.0 / C)
                musq = spool.tile([1, 512], F32, tag="sm512", bufs=6,
                                  name=f"{out_name}_musq{t}")
                nc.vector.tensor_mul(musq[:], mu[:], mu[:])
                nc.vector.tensor_sub(var[:], var[:], musq[:])
                sd = spool.tile([1, 512], F32, tag="sm512", bufs=6,
                                name=f"{out_name}_sd{t}")
                nc.scalar.activation(sd[:], var[:], AF.Sqrt, bias=eps_t[:])
                nc.vector.reciprocal(inv_t[:, ts(t, 512)], sd[:])
                nc.vector.tensor_mul(c0_t[:, ts(t, 512)], mu[:], inv_t[:, ts(t, 512)])
                nc.scalar.mul(c0_t[:, ts(t, 512)], c0_t[:, ts(t, 512)], -1.0)
            invb = spool.tile([P, T], F32R, tag="lnbc", name=f"{out_name}_invb")
            c0b = spool.tile([P, T], F32R, tag="lnbc", name=f"{out_name}_c0b")
            for t in range(NQ):
                for row, dst in ((inv_t, invb), (c0_t, c0b)):
                    bps = ps_mm.tile([P, 512], F32, tag="ps",
                                     name=f"{out_name}_bc{t}")
                    nc.tensor.matmul(bps[:], ones_row[:],
                                     row[:, ts(t, 512)], start=True, stop=True)
                    nc.scalar.activation(dst[:, ts(t, 512)], bps[:], AF.Copy)
            outs = []
            for c in range(NCH):
                h = hpool.tile([P, T], F32R, tag=out_tag, name=f"{out_name}{c}")
                nc.vector.tensor_mul(h[:], src[c][:], invb[:])
                nc.vector.tensor_add(h[:], h[:], c0b[:])
                nc.scalar.activation(h[:], h[:], AF.Identity,
                                     bias=b_t[:, c:c + 1], scale=g_t[:, c:c + 1])
                outs.append(h)
            return outs

        def linear_mtile(dst, w_src3, m, src_tiles, bias_col, func, nk=NCH,
                         wtag="w", name="lin"):
            """dst[:, :] (+bias, func) = W[:, m-chunk]^T @ src ; contraction nk*128."""
            wt = wpool.tile([P, nk, P], F32R, tag=wtag, bufs=3, name=f"{name}_w{m}")
            nc.sync.dma_start(wt[:], w_src3[:, :, ts(m, P)])
            for t in range(NQ):
                ps = ps_mm.tile([P, 512], F32, tag="ps", name=f"{name}_ps{m}_{t}")
                for j in range(nk):
                    nc.tensor.matmul(ps[:], wt[:, j, :],
                                     src_tiles[j][:, ts(t, 512)],
                                     start=(j == 0), stop=(j == nk - 1))
                nc.scalar.activation(dst[:, ts(t, 512)], ps[:], func,
                                     bias=bias_col, scale=1.0)

        # ---------------- load x (token-major) and transpose to FM ----------
        with tc.tile_pool(name="qkvt", bufs=6) as qkvt, \
             tc.tile_pool(name="vaug", bufs=12) as vaugp, \
             tc.tile_pool(name="ptp", bufs=3) as ptp, \
             tc.tile_pool(name="ypool", bufs=8) as ypool:

            xtm = [qkvt.tile([P, C], F32, tag="qkv", name=f"xtm{i}") for i in range(NT)]
            for i in range(NT):
                nc.sync.dma_start(xtm[i][:], x_d[ts(i, P), :])
            for i in range(NT):
                for m in range(NCH):
                    pst = ps_tr.tile([P, P], F32, tag="tr", name=f"xtr{i}_{m}")
                    nc.tensor.transpose(pst[:], xtm[i][:, ts(m, P)], ident[:])
                    nc.scalar.activation(x_t[m][:, ts(i, P)], pst[:], AF.Copy)

            # ---------------- LN1 ----------------
            h1 = layernorm_fm(x_t, ln1g_t, ln1b_t, "h", "h1")

            # ---------------- per-head-block QKV + attention ----------------
            y_t = [ypool.tile([P, T], F32R, tag="y", name=f"y{hb}")
                   for hb in range(NCH)]
            for hb in range(NCH):
                q_t = qkvt.tile([P, T], F32R, tag="qkv", name=f"q{hb}")
                k_t = qkvt.tile([P, T], F32R, tag="qkv", name=f"k{hb}")
                v_t = qkvt.tile([P, T], F32, tag="qkv", name=f"v{hb}")
                linear_mtile(q_t[:], Wqkv_r, hb, h1, bqkv_t[:, hb:hb + 1],
                             AF.Identity, name="q")
                linear_mtile(k_t[:], Wqkv_r, NCH + hb, h1,
                             bqkv_t[:, NCH + hb:NCH + hb + 1], AF.Identity, name="k")
                linear_mtile(v_t[:], Wqkv_r, 2 * NCH + hb, h1,
                             bqkv_t[:, 2 * NCH + hb:2 * NCH + hb + 1], AF.Identity,
                             name="v")
                # v -> token-major, per-head layout with a ones column:
                # vaug[ki] = [128(Tk), 130] : cols 0..63 head A, 64 ones,
                #                            65..128 head B, 129 ones
                vaug = [vaugp.tile([P, 130], F32R, tag="vaug", name=f"va{hb}_{ki}")
                        for ki in range(NT)]
                for ki in range(NT):
                    pst = ps_tr.tile([P, P], F32, tag="tr", name=f"vtr{hb}_{ki}")
                    nc.tensor.transpose(pst[:], v_t[:, ts(ki, P)], ident[:])
                    dst = vaug[ki][:].rearrange("p (h c) -> p h c", h=2)[:, :, 0:64]
                    src = pst[:].rearrange("p (h c) -> p h c", h=2)
                    nc.scalar.activation(dst, src, AF.Copy)
                    nc.scalar.activation(vaug[ki][:, 64:65], ones_f[:], AF.Copy)
                    nc.scalar.activation(vaug[ki][:, 129:130], ones_f[:], AF.Copy)
                for p_ in range(2):
                    qh = q_t[p_ * 64:(p_ + 1) * 64, :]
                    kh = k_t[p_ * 64:(p_ + 1) * 64, :]
                    for qi in range(NQ):
                        kmax = 4 * qi + 3
                        pv = ps_pv.tile([P, 512], F32, tag="pv",
                                        name=f"pv{hb}_{p_}_{qi}")
                        for ki in range(kmax + 1):
                            st = ps_mm.tile([P, 512], F32, tag="ps",
                                            name=f"st{hb}_{p_}_{qi}_{ki}")
                            nc.tensor.matmul(st[:], kh[:, ts(ki, P)],
                                             qh[:, ts(qi, 512)],
                                             start=True, stop=True)
                            pt = ptp.tile([P, 512], F32R, tag="pt",
                                          name=f"pt{hb}_{p_}_{qi}_{ki}")
                            nc.scalar.activation(pt[:], st[:], AF.Exp,
                                                 bias=zero_col[:], scale=SCALE)
                            if ki >= 4 * qi:  # partial (diagonal-band) block
                                nc.gpsimd.affine_select(
                                    out=pt[:], in_=pt[:],
                                    pattern=[[1, 512]],
                                    base=qi * 512 - ki * P,
                                    channel_multiplier=-1,
                                    compare_op=mybir.AluOpType.is_ge,
                                    fill=0.0)
                            nc.tensor.matmul(pv[0:65, :],
                                             vaug[ki][:, p_ * 65:(p_ + 1) * 65],
                                             pt[:],
                                             start=(ki == 0), stop=(ki == kmax))
                        dn = spool.tile([1, 512], F32R, tag="sm512",
                                        bufs=6, name=f"dn{hb}_{p_}_{qi}")
                        nc.vector.reciprocal(dn[:], pv[64:65, :])
                        dnb = spool.tile([64, 512], F32R, tag="dnb",
                                         bufs=4, name=f"dnb{hb}_{p_}_{qi}")
                        bps = ps_mm.tile([P, 512], F32, tag="ps",
                                         name=f"dnbc{hb}_{p_}_{qi}")
                        nc.tensor.matmul(bps[0:64, :], ones_row[:, 0:64],
                                         dn[:], start=True, stop=True)
                        nc.scalar.activation(dnb[:], bps[0:64, :], AF.Copy)
                        nc.vector.tensor_mul(
                            y_t[hb][p_ * 64:(p_ + 1) * 64, ts(qi, 512)],
                            pv[0:64, :], dnb[:])

            # ---------------- proj + residual (into x_t in place) -----------
            for m in range(NCH):
                nc.scalar.activation(x_t[m][:], x_t[m][:], AF.Identity,
                                     bias=bproj_t[:, m:m + 1], scale=1.0)
                wt = wpool.tile([P, NCH, P], F32R, tag="w", bufs=3,
                                name=f"proj_w{m}")
                nc.sync.dma_start(wt[:], Wproj_r[:, :, ts(m, P)])
                for t in range(NQ):
                    ps = ps_mm.tile([P, 512], F32, tag="ps", name=f"proj_ps{m}_{t}")
                    for j in range(NCH):
                        nc.tensor.matmul(ps[:], wt[:, j, :],
                                         y_t[j][:, ts(t, 512)],
                                         start=(j == 0), stop=(j == NCH - 1))
                    nc.vector.tensor_add(x_t[m][:, ts(t, 512)],
                                         x_t[m][:, ts(t, 512)], ps[:])

        # ---------------- LN2 ----------------
        h2 = layernorm_fm(x_t, ln2g_t, ln2b_t, "h", "h2")

        # ---------------- FFN (two d_ff halves) + residual ----------------
        with tc.tile_pool(name="a1pool", bufs=16) as a1pool:
            for m in range(NCH):  # pre-add b2 so FFN2 eviction is a plain add
                nc.scalar.activation(x_t[m][:], x_t[m][:], AF.Identity,
                                     bias=b2_t[:, m:m + 1], scale=1.0)
            for half in range(2):
                a1 = []
                for mm_ in range(16):
                    mg = half * 16 + mm_
                    a = a1pool.tile([P, T], F32R, tag="a1", name=f"a1_{mg}")
                    linear_mtile(a[:], W1_r, mg, h2, b1_t[:, mg:mg + 1],
                                 AF.Relu, name=f"ffn1_{mg}")
                    a1.append(a)
                for m in range(NCH):
                    w2t = wpool.tile([P, 16, P], F32R, tag="w2", name=f"w2_{half}_{m}")
                    nc.sync.dma_start(
                        w2t[:], W2_r[:, half * 16:(half + 1) * 16, ts(m, P)])
                    for t in range(NQ):
                        ps = ps_mm.tile([P, 512], F32, tag="ps",
                                        name=f"ffn2_ps{half}_{m}_{t}")
                        for j in range(16):
                            nc.tensor.matmul(ps[:], w2t[:, j, :],
                                             a1[j][:, ts(t, 512)],
                                             start=(j == 0), stop=(j == 15))
                        nc.vector.tensor_add(x_t[m][:, ts(t, 512)],
                                             x_t[m][:, ts(t, 512)], ps[:])

            # ---------------- transpose result back to token-major ----------
            for i in range(NT):
                ot = a1pool.tile([P, C], F32, tag="a1", name=f"ot{i}")
                for m in range(NCH):
                    pst = ps_tr.tile([P, P], F32R, tag="tr", name=f"otr{i}_{m}")
                    nc.tensor.transpose(pst[:], x_t[m][:, ts(i, P)], ident_r[:])
                    nc.scalar.activation(ot[:, ts(m, P)], pst[:], AF.Copy)
                nc.sync.dma_start(out_d[ts(i, P), :], ot[:])

    nc.compile()
    return nc


_NC_CACHE = {}


def _get_nc():
    if "nc" not in _NC_CACHE:
        _NC_CACHE["nc"] = _build()
    return _NC_CACHE["nc"]


def kernel(**inputs):
    from concourse.bass_utils import run_bass_kernel_spmd

    nc = _get_nc()
    names = ["Wqkv", "bqkv", "Wproj", "bproj", "ln1_g", "ln1_b", "ln2_g",
             "ln2_b", "W1", "b1", "W2", "b2"]
    shared = {k: np.ascontiguousarray(np.asarray(inputs[k], dtype=np.float32))
              for k in names}
    x = np.asarray(inputs["x"], dtype=np.float32)
    in_maps = [dict(shared, x=np.ascontiguousarray(x[i])) for i in range(B)]
    res = run_bass_kernel_spmd(nc, in_maps, core_ids=list(range(B)))
    out = np.stack([res.results[i]["out"] for i in range(B)], axis=0)
    return out.astype(np.float32)


# revision 16
# speedup vs baseline: 1.0011x; 1.0011x over previous
"""Trainium2 Bass kernel for one GPT-style transformer block.

Problem: x[8,1024,1024]; per-core = one batch element (data-parallel over 8
NeuronCores).  Per core:
    h1 = LN(x); qkv = h1@Wqkv+b; causal MHA (16 heads, d=64);
    r1 = x + attn@Wproj+b; h2 = LN(r1); out = r1 + relu(h2@W1+b1)@W2+b2

Design notes:
  - Activations live feature-major in SBUF: [C partition, T free], so every
    linear layer is matmul(lhsT=W[K=Cin,M=Cout], rhs=act[K=Cin,N=T]) with no
    activation transposes.
  - Matmuls run as float32r (TF32-like, full PE rate at N>=256).  The BIR
    verifier requires every f32r-matmul operand to be *produced* as f32r, so
    matmul-feeding tiles are typed float32r (engines round on write; DMA from
    f32r-typed DRAM passes through).  memset cannot write f32r -> constants
    are memset f32 then ACT-copied.
  - Attention computes S^T = (K-block)^T @ Q chunks -> [Tk,Tq] tiles; softmax
    denominators come for free by augmenting V^T with a ones column in the
    P@V matmul (row 64 of the PV psum = sum_k exp).  Causal masking via
    gpsimd.affine_select on the exp'd tiles.  No max-subtraction needed:
    |S| <= ~10 so exp never overflows fp32.
  - LN stats (sums over the partition axis) via ones-vector matmuls; the
    per-token scale/shift rows are broadcast across partitions with a rank-1
    (ones outer row) matmul.
  - FFN runs in two d_ff halves so the fp32 intermediate fits SBUF.

Measured on TRN2 (neuron-profile NTFF): 941 us/core, rel err 2.6e-4.
"""

import math
import sys

import numpy as np

sys.path.insert(0, "/opt/trn_rl_repo")

from contextlib import ExitStack

import concourse.bass as bass
import concourse.mybir as mybir
import concourse.tile as tile
from concourse import bacc
from concourse.bass import ts
from concourse.masks import make_identity

F32 = mybir.dt.float32
F32R = mybir.dt.float32r
AF = mybir.ActivationFunctionType

B, T, C, H = 8, 1024, 1024, 16
D = C // H
FF = 4 * C
P = 128
NCH = C // P          # 8 feature chunks
NT = T // P           # 8 token chunks of 128
NQ = T // 512         # 2 token chunks of 512
SCALE = 1.0 / math.sqrt(3 * C // H)
EPS = 1e-5


def _build():
    nc = bacc.Bacc("TRN2", target_bir_lowering=False, debug=False)

    x_d = nc.dram_tensor("x", [T, C], F32, kind="ExternalInput").ap()
    Wqkv_d = nc.dram_tensor("Wqkv", [C, 3 * C], F32R, kind="ExternalInput").ap()
    bqkv_d = nc.dram_tensor("bqkv", [3 * C], F32, kind="ExternalInput").ap()
    Wproj_d = nc.dram_tensor("Wproj", [C, C], F32R, kind="ExternalInput").ap()
    bproj_d = nc.dram_tensor("bproj", [C], F32, kind="ExternalInput").ap()
    ln1g_d = nc.dram_tensor("ln1_g", [C], F32, kind="ExternalInput").ap()
    ln1b_d = nc.dram_tensor("ln1_b", [C], F32, kind="ExternalInput").ap()
    ln2g_d = nc.dram_tensor("ln2_g", [C], F32, kind="ExternalInput").ap()
    ln2b_d = nc.dram_tensor("ln2_b", [C], F32, kind="ExternalInput").ap()
    W1_d = nc.dram_tensor("W1", [C, FF], F32R, kind="ExternalInput").ap()
    b1_d = nc.dram_tensor("b1", [FF], F32, kind="ExternalInput").ap()
    W2_d = nc.dram_tensor("W2", [FF, C], F32R, kind="ExternalInput").ap()
    b2_d = nc.dram_tensor("b2", [C], F32, kind="ExternalInput").ap()
    out_d = nc.dram_tensor("out", [T, C], F32, kind="ExternalOutput").ap()

    Wqkv_r = Wqkv_d.rearrange("(j p) m -> p j m", p=P)     # [128, 8, 3072]
    Wproj_r = Wproj_d.rearrange("(j p) m -> p j m", p=P)   # [128, 8, 1024]
    W1_r = W1_d.rearrange("(j p) m -> p j m", p=P)         # [128, 8, 4096]
    W2_r = W2_d.rearrange("(j p) m -> p j m", p=P)         # [128, 32, 1024]

    with nc.allow_low_precision(reason="fp32r matmul inputs (fp32 accum)"), \
         tile.TileContext(nc) as tc, ExitStack() as ctx:
        const = ctx.enter_context(tc.tile_pool(name="const", bufs=1))
        xpool = ctx.enter_context(tc.tile_pool(name="xpool", bufs=8))
        hpool = ctx.enter_context(tc.tile_pool(name="hpool", bufs=8))
        spool = ctx.enter_context(tc.tile_pool(name="spool", bufs=2))
        wpool = ctx.enter_context(tc.tile_pool(name="wpool", bufs=2))
        ps_mm = ctx.enter_context(tc.tile_pool(name="ps_mm", bufs=4, space="PSUM"))
        ps_pv = ctx.enter_context(tc.tile_pool(name="ps_pv", bufs=2, space="PSUM"))
        ps_tr = ctx.enter_context(tc.tile_pool(name="ps_tr", bufs=2, space="PSUM"))

        ident = const.tile([P, P], F32)
        make_identity(nc, ident[:])
        ident_r = const.tile([P, P], F32R)
        nc.scalar.activation(ident_r[:], ident[:], AF.Copy)
        ones_f = const.tile([P, 1], F32)
        nc.vector.memset(ones_f[:], 1.0)
        ones_col = const.tile([P, 1], F32R)
        nc.scalar.activation(ones_col[:], ones_f[:], AF.Copy)
        ones_rowf = const.tile([1, P], F32)
        nc.vector.memset(ones_rowf[:], 1.0)
        ones_row = const.tile([1, P], F32R)
        nc.scalar.activation(ones_row[:], ones_rowf[:], AF.Copy)
        eps_t = const.tile([1, 1], F32)
        nc.vector.memset(eps_t[:], EPS)
        zero_col = const.tile([P, 1], F32)
        nc.vector.memset(zero_col[:], 0.0)

        # bias/param columns: col m = vec[m*128:(m+1)*128]
        bqkv_t = const.tile([P, 3 * NCH], F32)
        nc.sync.dma_start(bqkv_t[:], bqkv_d.rearrange("(m p) -> p m", p=P))
        bproj_t = const.tile([P, NCH], F32)
        nc.sync.dma_start(bproj_t[:], bproj_d.rearrange("(m p) -> p m", p=P))
        b1_t = const.tile([P, FF // P], F32)
        nc.sync.dma_start(b1_t[:], b1_d.rearrange("(m p) -> p m", p=P))
        b2_t = const.tile([P, NCH], F32)
        nc.sync.dma_start(b2_t[:], b2_d.rearrange("(m p) -> p m", p=P))
        ln1g_t = const.tile([P, NCH], F32)
        nc.sync.dma_start(ln1g_t[:], ln1g_d.rearrange("(m p) -> p m", p=P))
        ln1b_t = const.tile([P, NCH], F32)
        nc.sync.dma_start(ln1b_t[:], ln1b_d.rearrange("(m p) -> p m", p=P))
        ln2g_t = const.tile([P, NCH], F32)
        nc.sync.dma_start(ln2g_t[:], ln2g_d.rearrange("(m p) -> p m", p=P))
        ln2b_t = const.tile([P, NCH], F32)
        nc.sync.dma_start(ln2b_t[:], ln2b_d.rearrange("(m p) -> p m", p=P))

        # persistent feature-major x tiles; become r1 then out in place
        x_t = [xpool.tile([P, T], F32R, tag="x", name=f"x_fm{m}") for m in range(NCH)]

        def layernorm_fm(src, g_t, b_t, out_tag, out_name):
            """src: 8 [128,1024] FM f32r tiles -> 8 normalized FM f32r tiles."""
            sum_ps = [ps_mm.tile([1, 512], F32, tag="ps", name=f"{out_name}_sum{t}")
                      for t in range(NQ)]
            sq_ps = [ps_mm.tile([1, 512], F32, tag="ps", name=f"{out_name}_sq{t}")
                     for t in range(NQ)]
            for c in range(NCH):
                for t in range(NQ):
                    sq = spool.tile([P, 512], F32R, tag="sq",
                                    name=f"{out_name}_sqv{c}_{t}")
                    nc.vector.tensor_mul(sq[:], src[c][:, ts(t, 512)],
                                         src[c][:, ts(t, 512)])
                    nc.tensor.matmul(
                        sum_ps[t][:], ones_col[:], src[c][:, ts(t, 512)],
                        start=(c == 0), stop=(c == NCH - 1))
                    nc.tensor.matmul(
                        sq_ps[t][:], ones_col[:], sq[:],
                        start=(c == 0), stop=(c == NCH - 1))
            inv_t = spool.tile([1, T], F32R, tag="lnstat", name=f"{out_name}_inv")
            c0_t = spool.tile([1, T], F32R, tag="lnstat", name=f"{out_name}_c0")
            for t in range(NQ):
                mu = spool.tile([1, 512], F32R, tag="sm512", bufs=6,
                                name=f"{out_name}_mu{t}")
                var = spool.tile([1, 512], F32, tag="sm512", bufs=6,
                                 name=f"{out_name}_var{t}")
                nc.scalar.mul(mu[:], sum_ps[t][:], 1.0 / C)
                nc.scalar.mul(var[:], sq_ps[t][:], 1.0 / C)
                musq = spool.tile([1, 512], F32, tag="sm512", bufs=6,
                                  name=f"{out_name}_musq{t}")
                nc.vector.tensor_mul(musq[:], mu[:], mu[:])
                nc.vector.tensor_sub(var[:], var[:], musq[:])
                sd = spool.tile([1, 512], F32, tag="sm512", bufs=6,
                                name=f"{out_name}_sd{t}")
                nc.scalar.activation(sd[:], var[:], AF.Sqrt, bias=eps_t[:])
                nc.vector.reciprocal(inv_t[:, ts(t, 512)], sd[:])
                nc.vector.tensor_mul(c0_t[:, ts(t, 512)], mu[:], inv_t[:, ts(t, 512)])
                nc.scalar.mul(c0_t[:, ts(t, 512)], c0_t[:, ts(t, 512)], -1.0)
            invb = spool.tile([P, T], F32R, tag="lnbc", name=f"{out_name}_invb")
            c0b = spool.tile([P, T], F32R, tag="lnbc", name=f"{out_name}_c0b")
            for t in range(NQ):
                for row, dst in ((inv_t, invb), (c0_t, c0b)):
                    bps = ps_mm.tile([P, 512], F32, tag="ps",
                                     name=f"{out_name}_bc{t}")
                    nc.tensor.matmul(bps[:], ones_row[:],
                                     row[:, ts(t, 512)], start=True, stop=True)
                    nc.scalar.activation(dst[:, ts(t, 512)], bps[:], AF.Copy)
            outs = []
            for c in range(NCH):
                h = hpool.tile([P, T], F32R, tag=out_tag, name=f"{out_name}{c}")
                nc.vector.tensor_mul(h[:], src[c][:], invb[:])
                nc.vector.tensor_add(h[:], h[:], c0b[:])
                nc.scalar.activation(h[:], h[:], AF.Identity,
                                     bias=b_t[:, c:c + 1], scale=g_t[:, c:c + 1])
                outs.append(h)
            return outs

        def linear_mtile(dst, w_src3, m, src_tiles, bias_col, func, nk=NCH,
                         wtag="w", name="lin"):
            """dst[:, :] (+bias, func) = W[:, m-chunk]^T @ src ; contraction nk*128."""
            wt = wpool.tile([P, nk, P], F32R, tag=wtag, bufs=3, name=f"{name}_w{m}")
            nc.sync.dma_start(wt[:], w_src3[:, :, ts(m, P)])
            for t in range(NQ):
                ps = ps_mm.tile([P, 512], F32, tag="ps", name=f"{name}_ps{m}_{t}")
                for j in range(nk):
                    nc.tensor.matmul(ps[:], wt[:, j, :],
                                     src_tiles[j][:, ts(t, 512)],
                                     start=(j == 0), stop=(j == nk - 1))
                nc.scalar.activation(dst[:, ts(t, 512)], ps[:], func,
                                     bias=bias_col, scale=1.0)

        # ---------------- load x (token-major) and transpose to FM ----------
        with tc.tile_pool(name="qkvt", bufs=6) as qkvt, \
             tc.tile_pool(name="vaug", bufs=12) as vaugp, \
             tc.tile_pool(name="ptp", bufs=3) as ptp, \
             tc.tile_pool(name="ypool", bufs=8) as ypool:

            xtm = [qkvt.tile([P, C], F32, tag="qkv", name=f"xtm{i}") for i in range(NT)]
            for i in range(NT):
                nc.sync.dma_start(xtm[i][:], x_d[ts(i, P), :])
            for i in range(NT):
                for m in range(NCH):
                    pst = ps_tr.tile([P, P], F32, tag="tr", name=f"xtr{i}_{m}")
                    nc.tensor.transpose(pst[:], xtm[i][:, ts(m, P)], ident[:])
                    nc.scalar.activation(x_t[m][:, ts(i, P)], pst[:], AF.Copy)

            # ---------------- LN1 ----------------
            h1 = layernorm_fm(x_t, ln1g_t, ln1b_t, "h", "h1")

            # ---------------- per-head-block QKV + attention ----------------
            y_t = [ypool.tile([P, T], F32R, tag="y", name=f"y{hb}")
                   for hb in range(NCH)]
            for hb in range(NCH):
                q_t = qkvt.tile([P, T], F32R, tag="qkv", name=f"q{hb}")
                k_t = qkvt.tile([P, T], F32R, tag="qkv", name=f"k{hb}")
                v_t = qkvt.tile([P, T], F32, tag="qkv", name=f"v{hb}")
                linear_mtile(q_t[:], Wqkv_r, hb, h1, bqkv_t[:, hb:hb + 1],
                             AF.Identity, name="q")
                linear_mtile(k_t[:], Wqkv_r, NCH + hb, h1,
                             bqkv_t[:, NCH + hb:NCH + hb + 1], AF.Identity, name="k")
                linear_mtile(v_t[:], Wqkv_r, 2 * NCH + hb, h1,
                             bqkv_t[:, 2 * NCH + hb:2 * NCH + hb + 1], AF.Identity,
                             name="v")
                # v -> token-major, per-head layout with a ones column:
                # vaug[ki] = [128(Tk), 130] : cols 0..63 head A, 64 ones,
                #                            65..128 head B, 129 ones
                vaug = [vaugp.tile([P, 130], F32R, tag="vaug", name=f"va{hb}_{ki}")
                        for ki in range(NT)]
                for ki in range(NT):
                    pst = ps_tr.tile([P, P], F32, tag="tr", name=f"vtr{hb}_{ki}")
                    nc.tensor.transpose(pst[:], v_t[:, ts(ki, P)], ident[:])
                    dst = vaug[ki][:].rearrange("p (h c) -> p h c", h=2)[:, :, 0:64]
                    src = pst[:].rearrange("p (h c) -> p h c", h=2)
                    nc.scalar.activation(dst, src, AF.Copy)
                    nc.scalar.activation(vaug[ki][:, 64:65], ones_f[:], AF.Copy)
                    nc.scalar.activation(vaug[ki][:, 129:130], ones_f[:], AF.Copy)
                for p_ in range(2):
                    qh = q_t[p_ * 64:(p_ + 1) * 64, :]
                    kh = k_t[p_ * 64:(p_ + 1) * 64, :]
                    for qi in range(NQ):
                        kmax = 4 * qi + 3
                        pv = ps_pv.tile([P, 512], F32, tag="pv",
                                        name=f"pv{hb}_{p_}_{qi}")
                        for ki in range(kmax + 1):
                            st = ps_mm.tile([P, 512], F32, tag="ps",
                                            name=f"st{hb}_{p_}_{qi}_{ki}")
                            nc.tensor.matmul(st[:], kh[:, ts(ki, P)],
                                             qh[:, ts(qi, 512)],
                                             start=True, stop=True)
                            pt = ptp.tile([P, 512], F32R, tag="pt",
                                          name=f"pt{hb}_{p_}_{qi}_{ki}")
                            nc.scalar.activation(pt[:], st[:], AF.Exp,
                                                 bias=zero_col[:], scale=SCALE)
                            if ki >= 4 * qi:  # partial (diagonal-band) block
                                nc.gpsimd.affine_select(
                                    out=pt[:], in_=pt[:],
                                    pattern=[[1, 512]],
                                    base=qi * 512 - ki * P,
                                    channel_multiplier=-1,
                                    compare_op=mybir.AluOpType.is_ge,
                                    fill=0.0)
                            nc.tensor.matmul(pv[0:65, :],
                                             vaug[ki][:, p_ * 65:(p_ + 1) * 65],
                                             pt[:],
                                             start=(ki == 0), stop=(ki == kmax))
                        dn = spool.tile([1, 512], F32R, tag="sm512",
                                        bufs=6, name=f"dn{hb}_{p_}_{qi}")
                        nc.vector.reciprocal(dn[:], pv[64:65, :])
                        dnb = spool.tile([64, 512], F32R, tag="dnb",
                                         bufs=4, name=f"dnb{hb}_{p_}_{qi}")
                        bps = ps_mm.tile([P, 512], F32, tag="ps",
                                         name=f"dnbc{hb}_{p_}_{qi}")
                        nc.tensor.matmul(bps[0:64, :], ones_row[:, 0:64],
                                         dn[:], start=True, stop=True)
                        nc.scalar.activation(dnb[:], bps[0:64, :], AF.Copy)
                        nc.vector.tensor_mul(
                            y_t[hb][p_ * 64:(p_ + 1) * 64, ts(qi, 512)],
                            pv[0:64, :], dnb[:])

            # ---------------- proj + residual (into x_t in place) -----------
            for m in range(NCH):
                nc.scalar.activation(x_t[m][:], x_t[m][:], AF.Identity,
                                     bias=bproj_t[:, m:m + 1], scale=1.0)
                wt = wpool.tile([P, NCH, P], F32R, tag="w", bufs=3,
                                name=f"proj_w{m}")
                nc.sync.dma_start(wt[:], Wproj_r[:, :, ts(m, P)])
                for t in range(NQ):
                    ps = ps_mm.tile([P, 512], F32, tag="ps", name=f"proj_ps{m}_{t}")
                    for j in range(NCH):
                        nc.tensor.matmul(ps[:], wt[:, j, :],
                                         y_t[j][:, ts(t, 512)],
                                         start=(j == 0), stop=(j == NCH - 1))
                    nc.vector.tensor_add(x_t[m][:, ts(t, 512)],
                                         x_t[m][:, ts(t, 512)], ps[:])

        # ---------------- LN2 ----------------
        h2 = layernorm_fm(x_t, ln2g_t, ln2b_t, "h", "h2")

        # ---------------- FFN (two d_ff halves) + residual ----------------
        with tc.tile_pool(name="a1pool", bufs=16) as a1pool:
            for m in range(NCH):  # pre-add b2 so FFN2 eviction is a plain add
                nc.scalar.activation(x_t[m][:], x_t[m][:], AF.Identity,
                                     bias=b2_t[:, m:m + 1], scale=1.0)
            for half in range(2):
                a1 = []
                for mm_ in range(16):
                    mg = half * 16 + mm_
                    a = a1pool.tile([P, T], F32R, tag="a1", name=f"a1_{mg}")
                    linear_mtile(a[:], W1_r, mg, h2, b1_t[:, mg:mg + 1],
                                 AF.Relu, name=f"ffn1_{mg}")
                    a1.append(a)
                for m in range(NCH):
                    w2t = wpool.tile([P, 16, P], F32R, tag="w2", name=f"w2_{half}_{m}")
                    nc.sync.dma_start(
                        w2t[:], W2_r[:, half * 16:(half + 1) * 16, ts(m, P)])
                    for t in range(NQ):
                        ps = ps_mm.tile([P, 512], F32, tag="ps",
                                        name=f"ffn2_ps{half}_{m}_{t}")
                        for j in range(16):
                            nc.tensor.matmul(ps[:], w2t[:, j, :],
                                             a1[j][:, ts(t, 512)],
                                             start=(j == 0), stop=(j == 15))
                        nc.vector.tensor_add(x_t[m][:, ts(t, 512)],
                                             x_t[m][:, ts(t, 512)], ps[:])

            # ---------------- transpose result back to token-major ----------
            for i in range(NT):
                ot = a1pool.tile([P, C], F32, tag="a1", name=f"ot{i}")
                for m in range(NCH):
                    pst = ps_tr.tile([P, P], F32R, tag="tr", name=f"otr{i}_{m}")
                    nc.tensor.transpose(pst[:], x_t[m][:, ts(i, P)], ident_r[:])
                    nc.scalar.activation(ot[:, ts(m, P)], pst[:], AF.Copy)
                nc.sync.dma_start(out_d[ts(i, P), :], ot[:])

    nc.compile()
    return nc


_NC_CACHE = {}


def _get_nc():
    if "nc" not in _NC_CACHE:
        _NC_CACHE["nc"] = _build()
    return _NC_CACHE["nc"]


def kernel(**inputs):
    from concourse.bass_utils import run_bass_kernel_spmd

    nc = _get_nc()
    names = ["Wqkv", "bqkv", "Wproj", "bproj", "ln1_g", "ln1_b", "ln2_g",
             "ln2_b", "W1", "b1", "W2", "b2"]
    shared = {k: np.ascontiguousarray(np.asarray(inputs[k], dtype=np.float32))
              for k in names}
    x = np.asarray(inputs["x"], dtype=np.float32)
    in_maps = [dict(shared, x=np.ascontiguousarray(x[i])) for i in range(B)]
    res = run_bass_kernel_spmd(nc, in_maps, core_ids=list(range(B)))
    out = np.stack([res.results[i]["out"] for i in range(B)], axis=0)
    return out.astype(np.float32)


# revision 25
# speedup vs baseline: 1.0426x; 1.0415x over previous
"""Trainium2 Bass kernel for one GPT-style transformer block.

Problem: x[8,1024,1024]; per-core = one batch element (data-parallel over 8
NeuronCores).  Per core:
    h1 = LN(x); qkv = h1@Wqkv+b; causal MHA (16 heads, d=64);
    r1 = x + attn@Wproj+b; h2 = LN(r1); out = r1 + relu(h2@W1+b1)@W2+b2

Design notes:
  - Activations live feature-major in SBUF: [C partition, T free], so every
    linear layer is matmul(lhsT=W[K=Cin,M=Cout], rhs=act[K=Cin,N=T]) with no
    activation transposes.
  - Matmuls run as float32r (TF32-like, full PE rate at N>=256).  The BIR
    verifier requires every f32r-matmul operand to be *produced* as f32r, so
    matmul-feeding tiles are typed float32r (engines round on write; DMA from
    f32r-typed DRAM passes through).  memset cannot write f32r -> constants
    are memset f32 then ACT-copied.
  - Attention computes S^T = (K-block)^T @ Q chunks -> [Tk,Tq] tiles; softmax
    denominators come for free by augmenting V^T with a ones column in the
    P@V matmul (row 64 of the PV psum = sum_k exp).  Causal masking via
    gpsimd.affine_select on the exp'd tiles.  No max-subtraction needed:
    |S| <= ~10 so exp never overflows fp32.
  - LN stats (sums over the partition axis) via ones-vector matmuls; the
    per-token scale/shift rows are broadcast across partitions with a rank-1
    (ones outer row) matmul.
  - FFN runs in two d_ff halves so the fp32 intermediate fits SBUF.

Measured on TRN2 (neuron-profile NTFF): 941 us/core, rel err 2.6e-4.
"""

import math
import sys

import numpy as np

sys.path.insert(0, "/opt/trn_rl_repo")

from contextlib import ExitStack

import concourse.bass as bass
import concourse.mybir as mybir
import concourse.tile as tile
from concourse import bacc
from concourse.bass import ts
from concourse.masks import make_identity

F32 = mybir.dt.float32
F32R = mybir.dt.float32r
AF = mybir.ActivationFunctionType

B, T, C, H = 8, 1024, 1024, 16
D = C // H
FF = 4 * C
P = 128
NCH = C // P          # 8 feature chunks
NT = T // P           # 8 token chunks of 128
NQ = T // 512         # 2 token chunks of 512
SCALE = 1.0 / math.sqrt(3 * C // H)
EPS = 1e-5


def _build():
    nc = bacc.Bacc("TRN2", target_bir_lowering=False, debug=False)

    x_d = nc.dram_tensor("x", [T, C], F32, kind="ExternalInput").ap()
    Wqkv_d = nc.dram_tensor("Wqkv", [C, 3 * C], F32R, kind="ExternalInput").ap()
    bqkv_d = nc.dram_tensor("bqkv", [3 * C], F32, kind="ExternalInput").ap()
    Wproj_d = nc.dram_tensor("Wproj", [C, C], F32R, kind="ExternalInput").ap()
    bproj_d = nc.dram_tensor("bproj", [C], F32, kind="ExternalInput").ap()
    ln1g_d = nc.dram_tensor("ln1_g", [C], F32, kind="ExternalInput").ap()
    ln1b_d = nc.dram_tensor("ln1_b", [C], F32, kind="ExternalInput").ap()
    ln2g_d = nc.dram_tensor("ln2_g", [C], F32, kind="ExternalInput").ap()
    ln2b_d = nc.dram_tensor("ln2_b", [C], F32, kind="ExternalInput").ap()
    W1_d = nc.dram_tensor("W1", [C, FF], F32R, kind="ExternalInput").ap()
    b1_d = nc.dram_tensor("b1", [FF], F32, kind="ExternalInput").ap()
    W2_d = nc.dram_tensor("W2", [FF, C], F32R, kind="ExternalInput").ap()
    b2_d = nc.dram_tensor("b2", [C], F32, kind="ExternalInput").ap()
    out_d = nc.dram_tensor("out", [T, C], F32, kind="ExternalOutput").ap()

    Wqkv_r = Wqkv_d.rearrange("(j p) m -> p j m", p=P)     # [128, 8, 3072]
    Wproj_r = Wproj_d.rearrange("(j p) m -> p j m", p=P)   # [128, 8, 1024]
    W1_r = W1_d.rearrange("(j p) m -> p j m", p=P)         # [128, 8, 4096]
    W2_r = W2_d.rearrange("(j p) m -> p j m", p=P)         # [128, 32, 1024]

    with nc.allow_low_precision(reason="fp32r matmul inputs (fp32 accum)"), \
         tile.TileContext(nc) as tc, ExitStack() as ctx:
        const = ctx.enter_context(tc.tile_pool(name="const", bufs=1))
        xpool = ctx.enter_context(tc.tile_pool(name="xpool", bufs=8))
        hpool = ctx.enter_context(tc.tile_pool(name="hpool", bufs=8))
        spool = ctx.enter_context(tc.tile_pool(name="spool", bufs=2))
        wpool = ctx.enter_context(tc.tile_pool(name="wpool", bufs=2))
        ps_mm = ctx.enter_context(tc.tile_pool(name="ps_mm", bufs=4, space="PSUM"))
        ps_pv = ctx.enter_context(tc.tile_pool(name="ps_pv", bufs=2, space="PSUM"))
        ps_tr = ctx.enter_context(tc.tile_pool(name="ps_tr", bufs=2, space="PSUM"))

        ident = const.tile([P, P], F32)
        make_identity(nc, ident[:])
        ident_r = const.tile([P, P], F32R)
        nc.scalar.activation(ident_r[:], ident[:], AF.Copy)
        ones_f = const.tile([P, 1], F32)
        nc.vector.memset(ones_f[:], 1.0)
        ones_col = const.tile([P, 1], F32R)
        nc.scalar.activation(ones_col[:], ones_f[:], AF.Copy)
        ones_rowf = const.tile([1, P], F32)
        nc.vector.memset(ones_rowf[:], 1.0)
        ones_row = const.tile([1, P], F32R)
        nc.scalar.activation(ones_row[:], ones_rowf[:], AF.Copy)
        eps_t = const.tile([1, 1], F32)
        nc.vector.memset(eps_t[:], EPS)
        zero_col = const.tile([P, 1], F32)
        nc.vector.memset(zero_col[:], 0.0)

        # causal masks for diagonal-band blocks: mask_d[r,c] = 1 if c-r >= d*128
        masks = []
        with tc.tile_pool(name="mbuild", bufs=4) as mbp:
            for di in range(4):
                mf = mbp.tile([P, 512], F32, tag="mf", name=f"mf{di}")
                nc.gpsimd.memset(mf[:], 1.0)
                nc.gpsimd.affine_select(
                    out=mf[:], in_=mf[:], pattern=[[1, 512]],
                    base=-di * P, channel_multiplier=-1,
                    compare_op=mybir.AluOpType.is_ge, fill=0.0)
                mk = const.tile([P, 512], F32R, tag=f"mask{di}", name=f"mask{di}")
                nc.scalar.activation(mk[:], mf[:], AF.Copy)
                masks.append(mk)

        # bias/param columns: col m = vec[m*128:(m+1)*128]
        bqkv_t = const.tile([P, 3 * NCH], F32)
        nc.sync.dma_start(bqkv_t[:], bqkv_d.rearrange("(m p) -> p m", p=P))
        bproj_t = const.tile([P, NCH], F32)
        nc.sync.dma_start(bproj_t[:], bproj_d.rearrange("(m p) -> p m", p=P))
        b1_t = const.tile([P, FF // P], F32)
        nc.sync.dma_start(b1_t[:], b1_d.rearrange("(m p) -> p m", p=P))
        b2_t = const.tile([P, NCH], F32)
        nc.sync.dma_start(b2_t[:], b2_d.rearrange("(m p) -> p m", p=P))
        ln1g_t = const.tile([P, NCH], F32)
        nc.sync.dma_start(ln1g_t[:], ln1g_d.rearrange("(m p) -> p m", p=P))
        ln1b_t = const.tile([P, NCH], F32)
        nc.sync.dma_start(ln1b_t[:], ln1b_d.rearrange("(m p) -> p m", p=P))
        ln2g_t = const.tile([P, NCH], F32)
        nc.sync.dma_start(ln2g_t[:], ln2g_d.rearrange("(m p) -> p m", p=P))
        ln2b_t = const.tile([P, NCH], F32)
        nc.sync.dma_start(ln2b_t[:], ln2b_d.rearrange("(m p) -> p m", p=P))

        # persistent feature-major x tiles; become r1 then out in place
        x_t = [xpool.tile([P, T], F32R, tag="x", name=f"x_fm{m}") for m in range(NCH)]

        def layernorm_fm(src, g_t, b_t, out_tag, out_name):
            """src: 8 [128,1024] FM f32r tiles -> 8 normalized FM f32r tiles."""
            sum_ps = [ps_mm.tile([1, 512], F32, tag="ps", name=f"{out_name}_sum{t}")
                      for t in range(NQ)]
            sq_ps = [ps_mm.tile([1, 512], F32, tag="ps", name=f"{out_name}_sq{t}")
                     for t in range(NQ)]
            for c in range(NCH):
                for t in range(NQ):
                    sq = spool.tile([P, 512], F32R, tag="sq",
                                    name=f"{out_name}_sqv{c}_{t}")
                    nc.vector.tensor_mul(sq[:], src[c][:, ts(t, 512)],
                                         src[c][:, ts(t, 512)])
                    nc.tensor.matmul(
                        sum_ps[t][:], ones_col[:], src[c][:, ts(t, 512)],
                        start=(c == 0), stop=(c == NCH - 1))
                    nc.tensor.matmul(
                        sq_ps[t][:], ones_col[:], sq[:],
                        start=(c == 0), stop=(c == NCH - 1))
            inv_t = spool.tile([1, T], F32R, tag="lnstat", name=f"{out_name}_inv")
            c0_t = spool.tile([1, T], F32R, tag="lnstat", name=f"{out_name}_c0")
            for t in range(NQ):
                mu = spool.tile([1, 512], F32R, tag="sm512", bufs=5,
                                name=f"{out_name}_mu{t}")
                var = spool.tile([1, 512], F32, tag="sm512", bufs=5,
                                 name=f"{out_name}_var{t}")
                nc.scalar.mul(mu[:], sum_ps[t][:], 1.0 / C)
                nc.scalar.mul(var[:], sq_ps[t][:], 1.0 / C)
                musq = spool.tile([1, 512], F32, tag="sm512", bufs=5,
                                  name=f"{out_name}_musq{t}")
                nc.vector.tensor_mul(musq[:], mu[:], mu[:])
                nc.vector.tensor_sub(var[:], var[:], musq[:])
                sd = spool.tile([1, 512], F32, tag="sm512", bufs=5,
                                name=f"{out_name}_sd{t}")
                nc.scalar.activation(sd[:], var[:], AF.Sqrt, bias=eps_t[:])
                nc.vector.reciprocal(inv_t[:, ts(t, 512)], sd[:])
                nc.vector.tensor_mul(c0_t[:, ts(t, 512)], mu[:], inv_t[:, ts(t, 512)])
                nc.scalar.mul(c0_t[:, ts(t, 512)], c0_t[:, ts(t, 512)], -1.0)
            invb = spool.tile([P, T], F32R, tag="lnbc", name=f"{out_name}_invb")
            c0b = spool.tile([P, T], F32R, tag="lnbc", name=f"{out_name}_c0b")
            for t in range(NQ):
                for row, dst in ((inv_t, invb), (c0_t, c0b)):
                    bps = ps_mm.tile([P, 512], F32, tag="ps",
                                     name=f"{out_name}_bc{t}")
                    nc.tensor.matmul(bps[:], ones_row[:],
                                     row[:, ts(t, 512)], start=True, stop=True)
                    nc.scalar.activation(dst[:, ts(t, 512)], bps[:], AF.Copy)
            outs = []
            for c in range(NCH):
                h = hpool.tile([P, T], F32R, tag=out_tag, name=f"{out_name}{c}")
                nc.vector.tensor_mul(h[:], src[c][:], invb[:])
                nc.vector.tensor_add(h[:], h[:], c0b[:])
                nc.scalar.activation(h[:], h[:], AF.Identity,
                                     bias=b_t[:, c:c + 1], scale=g_t[:, c:c + 1])
                outs.append(h)
            return outs

        def linear_mtile(dst, w_src3, m, src_tiles, bias_col, func, nk=NCH,
                         wtag="w", name="lin"):
            """dst[:, :] (+bias, func) = W[:, m-chunk]^T @ src ; contraction nk*128."""
            wt = wpool.tile([P, nk, P], F32R, tag=wtag, bufs=3, name=f"{name}_w{m}")
            nc.sync.dma_start(wt[:], w_src3[:, :, ts(m, P)])
            for t in range(NQ):
                ps = ps_mm.tile([P, 512], F32, tag="ps", name=f"{name}_ps{m}_{t}")
                for j in range(nk):
                    nc.tensor.matmul(ps[:], wt[:, j, :],
                                     src_tiles[j][:, ts(t, 512)],
                                     start=(j == 0), stop=(j == nk - 1))
                nc.scalar.activation(dst[:, ts(t, 512)], ps[:], func,
                                     bias=bias_col, scale=1.0)

        # ---------------- load x (token-major) and transpose to FM ----------
        with tc.tile_pool(name="qkvt", bufs=6) as qkvt, \
             tc.tile_pool(name="vaug", bufs=10) as vaugp, \
             tc.tile_pool(name="ptp", bufs=4) as ptp, \
             tc.tile_pool(name="ypool", bufs=8) as ypool:

            xtm = [qkvt.tile([P, C], F32, tag="qkv", name=f"xtm{i}") for i in range(NT)]
            for i in range(NT):
                nc.sync.dma_start(xtm[i][:], x_d[ts(i, P), :])
            for i in range(NT):
                for m in range(NCH):
                    pst = ps_tr.tile([P, P], F32, tag="tr", name=f"xtr{i}_{m}")
                    nc.tensor.transpose(pst[:], xtm[i][:, ts(m, P)], ident[:])
                    nc.scalar.activation(x_t[m][:, ts(i, P)], pst[:], AF.Copy)

            # ---------------- LN1 ----------------
            h1 = layernorm_fm(x_t, ln1g_t, ln1b_t, "h", "h1")

            # ---------------- per-head-block QKV + attention ----------------
            y_t = [ypool.tile([P, T], F32R, tag="y", name=f"y{hb}")
                   for hb in range(NCH)]
            for hb in range(NCH):
                q_t = qkvt.tile([P, T], F32R, tag="qkv", name=f"q{hb}")
                k_t = qkvt.tile([P, T], F32R, tag="qkv", name=f"k{hb}")
                v_t = qkvt.tile([P, T], F32, tag="qkv", name=f"v{hb}")
                linear_mtile(q_t[:], Wqkv_r, hb, h1, bqkv_t[:, hb:hb + 1],
                             AF.Identity, name="q")
                linear_mtile(k_t[:], Wqkv_r, NCH + hb, h1,
                             bqkv_t[:, NCH + hb:NCH + hb + 1], AF.Identity, name="k")
                linear_mtile(v_t[:], Wqkv_r, 2 * NCH + hb, h1,
                             bqkv_t[:, 2 * NCH + hb:2 * NCH + hb + 1], AF.Identity,
                             name="v")
                # v -> token-major, per-head layout with a ones column:
                # vaug[ki] = [128(Tk), 130] : cols 0..63 head A, 64 ones,
                #                            65..128 head B, 129 ones
                vaug = [vaugp.tile([P, 130], F32R, tag="vaug", name=f"va{hb}_{ki}")
                        for ki in range(NT)]
                for ki in range(NT):
                    pst = ps_tr.tile([P, P], F32, tag="tr", name=f"vtr{hb}_{ki}")
                    nc.tensor.transpose(pst[:], v_t[:, ts(ki, P)], ident[:])
                    dst = vaug[ki][:].rearrange("p (h c) -> p h c", h=2)[:, :, 0:64]
                    src = pst[:].rearrange("p (h c) -> p h c", h=2)
                    nc.scalar.activation(dst, src, AF.Copy)
                    nc.scalar.activation(vaug[ki][:, 64:65], ones_f[:], AF.Copy)
                    nc.scalar.activation(vaug[ki][:, 129:130], ones_f[:], AF.Copy)
                for p_ in range(2):
                    qh = q_t[p_ * 64:(p_ + 1) * 64, :]
                    kh = k_t[p_ * 64:(p_ + 1) * 64, :]
                    for qi in range(NQ):
                        kmax = 4 * qi + 3
                        pv = ps_pv.tile([P, 512], F32, tag="pv",
                                        name=f"pv{hb}_{p_}_{qi}")
                        for ki in range(kmax + 1):
                            st = ps_mm.tile([P, 512], F32, tag="ps",
                                            name=f"st{hb}_{p_}_{qi}_{ki}")
                            nc.tensor.matmul(st[:], kh[:, ts(ki, P)],
                                             qh[:, ts(qi, 512)],
                                             start=True, stop=True)
                            pt = ptp.tile([P, 512], F32R, tag="pt",
                                          name=f"pt{hb}_{p_}_{qi}_{ki}")
                            nc.scalar.activation(pt[:], st[:], AF.Exp,
                                                 bias=zero_col[:], scale=SCALE)
                            if ki >= 4 * qi:  # diagonal-band block: DVE mask
                                ptm = ptp.tile([P, 512], F32R, tag="pt",
                                               name=f"ptm{hb}_{p_}_{qi}_{ki}")
                                nc.vector.tensor_mul(ptm[:], pt[:],
                                                     masks[ki - 4 * qi][:])
                                pt = ptm
                            nc.tensor.matmul(pv[0:65, :],
                                             vaug[ki][:, p_ * 65:(p_ + 1) * 65],
                                             pt[:],
                                             start=(ki == 0), stop=(ki == kmax))
                        dnrow = spool.tile([1, 512], F32, tag="sm512",
                                           bufs=5, name=f"dr{hb}_{p_}_{qi}")
                        nc.scalar.activation(dnrow[:], pv[64:65, :], AF.Copy)
                        dn = spool.tile([1, 512], F32, tag="sm512",
                                        bufs=5, name=f"dn{hb}_{p_}_{qi}")
                        nc.vector.reciprocal_approx_fast(dn[:], dnrow[:])
                        dnr = spool.tile([1, 512], F32R, tag="sm512",
                                         bufs=5, name=f"dq{hb}_{p_}_{qi}")
                        nc.scalar.activation(dnr[:], dn[:], AF.Copy)
                        dnb = spool.tile([64, 512], F32R, tag="dnb",
                                         bufs=2, name=f"dnb{hb}_{p_}_{qi}")
                        bps = ps_mm.tile([P, 512], F32, tag="ps",
                                         name=f"dnbc{hb}_{p_}_{qi}")
                        nc.tensor.matmul(bps[0:64, :], ones_row[:, 0:64],
                                         dnr[:], start=True, stop=True)
                        nc.scalar.activation(dnb[:], bps[0:64, :], AF.Copy)
                        nc.vector.tensor_mul(
                            y_t[hb][p_ * 64:(p_ + 1) * 64, ts(qi, 512)],
                            pv[0:64, :], dnb[:])

            # ---------------- proj + residual (into x_t in place) -----------
            for m in range(NCH):
                nc.scalar.activation(x_t[m][:], x_t[m][:], AF.Identity,
                                     bias=bproj_t[:, m:m + 1], scale=1.0)
                wt = wpool.tile([P, NCH, P], F32R, tag="w", bufs=3,
                                name=f"proj_w{m}")
                nc.sync.dma_start(wt[:], Wproj_r[:, :, ts(m, P)])
                for t in range(NQ):
                    ps = ps_mm.tile([P, 512], F32, tag="ps", name=f"proj_ps{m}_{t}")
                    for j in range(NCH):
                        nc.tensor.matmul(ps[:], wt[:, j, :],
                                         y_t[j][:, ts(t, 512)],
                                         start=(j == 0), stop=(j == NCH - 1))
                    nc.vector.tensor_add(x_t[m][:, ts(t, 512)],
                                         x_t[m][:, ts(t, 512)], ps[:])

        # ---------------- LN2 ----------------
        h2 = layernorm_fm(x_t, ln2g_t, ln2b_t, "h", "h2")

        # ---------------- FFN (two d_ff halves) + residual ----------------
        with tc.tile_pool(name="a1pool", bufs=16) as a1pool:
            for m in range(NCH):  # pre-add b2 so FFN2 eviction is a plain add
                nc.scalar.activation(x_t[m][:], x_t[m][:], AF.Identity,
                                     bias=b2_t[:, m:m + 1], scale=1.0)
            for half in range(2):
                a1 = []
                for mm_ in range(16):
                    mg = half * 16 + mm_
                    a = a1pool.tile([P, T], F32R, tag="a1", name=f"a1_{mg}")
                    linear_mtile(a[:], W1_r, mg, h2, b1_t[:, mg:mg + 1],
                                 AF.Relu, name=f"ffn1_{mg}")
                    a1.append(a)
                for m in range(NCH):
                    w2t = wpool.tile([P, 16, P], F32R, tag="w2", name=f"w2_{half}_{m}")
                    nc.sync.dma_start(
                        w2t[:], W2_r[:, half * 16:(half + 1) * 16, ts(m, P)])
                    for t in range(NQ):
                        ps = ps_mm.tile([P, 512], F32, tag="ps",
                                        name=f"ffn2_ps{half}_{m}_{t}")
                        for j in range(16):
                            nc.tensor.matmul(ps[:], w2t[:, j, :],
                                             a1[j][:, ts(t, 512)],
                                             start=(j == 0), stop=(j == 15))
                        nc.vector.tensor_add(x_t[m][:, ts(t, 512)],
                                             x_t[m][:, ts(t, 512)], ps[:])

            # ---------------- transpose result back to token-major ----------
            for i in range(NT):
                ot = a1pool.tile([P, C], F32, tag="a1", name=f"ot{i}")
                for m in range(NCH):
                    pst = ps_tr.tile([P, P], F32R, tag="tr", name=f"otr{i}_{m}")
                    nc.tensor.transpose(pst[:], x_t[m][:, ts(i, P)], ident_r[:])
                    nc.scalar.activation(ot[:, ts(m, P)], pst[:], AF.Copy)
                nc.sync.dma_start(out_d[ts(i, P), :], ot[:])

    nc.compile()
    return nc


_NC_CACHE = {}


def _get_nc():
    if "nc" not in _NC_CACHE:
        _NC_CACHE["nc"] = _build()
    return _NC_CACHE["nc"]


def kernel(**inputs):
    from concourse.bass_utils import run_bass_kernel_spmd

    nc = _get_nc()
    names = ["Wqkv", "bqkv", "Wproj", "bproj", "ln1_g", "ln1_b", "ln2_g",
             "ln2_b", "W1", "b1", "W2", "b2"]
    shared = {k: np.ascontiguousarray(np.asarray(inputs[k], dtype=np.float32))
              for k in names}
    x = np.asarray(inputs["x"], dtype=np.float32)
    in_maps = [dict(shared, x=np.ascontiguousarray(x[i])) for i in range(B)]
    res = run_bass_kernel_spmd(nc, in_maps, core_ids=list(range(B)))
    out = np.stack([res.results[i]["out"] for i in range(B)], axis=0)
    return out.astype(np.float32)


# revision 27
# speedup vs baseline: 1.0934x; 1.0487x over previous
"""Trainium2 Bass kernel for one GPT-style transformer block.

Problem: x[8,1024,1024]; per-core = one batch element (data-parallel over 8
NeuronCores).  Per core:
    h1 = LN(x); qkv = h1@Wqkv+b; causal MHA (16 heads, d=64);
    r1 = x + attn@Wproj+b; h2 = LN(r1); out = r1 + relu(h2@W1+b1)@W2+b2

Design notes:
  - Activations live feature-major in SBUF: [C partition, T free], so every
    linear layer is matmul(lhsT=W[K=Cin,M=Cout], rhs=act[K=Cin,N=T]) with no
    activation transposes.
  - Matmuls run as float32r (TF32-like, full PE rate at N>=256).  The BIR
    verifier requires every f32r-matmul operand to be *produced* as f32r, so
    matmul-feeding tiles are typed float32r (engines round on write; DMA from
    f32r-typed DRAM passes through).  memset cannot write f32r -> constants
    are memset f32 then ACT-copied.
  - Attention computes S^T = (K-block)^T @ Q chunks -> [Tk,Tq] tiles; softmax
    denominators come for free by augmenting V^T with a ones column in the
    P@V matmul (row 64 of the PV psum = sum_k exp).  Causal masking via
    gpsimd.affine_select on the exp'd tiles.  No max-subtraction needed:
    |S| <= ~10 so exp never overflows fp32.
  - LN stats (sums over the partition axis) via ones-vector matmuls; the
    per-token scale/shift rows are broadcast across partitions with a rank-1
    (ones outer row) matmul.
  - FFN runs in two d_ff halves so the fp32 intermediate fits SBUF.

Measured on TRN2 (neuron-profile NTFF): 903 us/core, rel err 2.6e-4.
"""

import math
import sys

import numpy as np

sys.path.insert(0, "/opt/trn_rl_repo")

from contextlib import ExitStack

import concourse.bass as bass
import concourse.mybir as mybir
import concourse.tile as tile
from concourse import bacc
from concourse.bass import ts
from concourse.masks import make_identity

F32 = mybir.dt.float32
F32R = mybir.dt.float32r
AF = mybir.ActivationFunctionType

B, T, C, H = 8, 1024, 1024, 16
D = C // H
FF = 4 * C
P = 128
NCH = C // P          # 8 feature chunks
NT = T // P           # 8 token chunks of 128
NQ = T // 512         # 2 token chunks of 512
SCALE = 1.0 / math.sqrt(3 * C // H)
EPS = 1e-5


def _build():
    nc = bacc.Bacc("TRN2", target_bir_lowering=False, debug=False)

    x_d = nc.dram_tensor("x", [T, C], F32, kind="ExternalInput").ap()
    Wqkv_d = nc.dram_tensor("Wqkv", [C, 3 * C], F32R, kind="ExternalInput").ap()
    bqkv_d = nc.dram_tensor("bqkv", [3 * C], F32, kind="ExternalInput").ap()
    Wproj_d = nc.dram_tensor("Wproj", [C, C], F32R, kind="ExternalInput").ap()
    bproj_d = nc.dram_tensor("bproj", [C], F32, kind="ExternalInput").ap()
    ln1g_d = nc.dram_tensor("ln1_g", [C], F32, kind="ExternalInput").ap()
    ln1b_d = nc.dram_tensor("ln1_b", [C], F32, kind="ExternalInput").ap()
    ln2g_d = nc.dram_tensor("ln2_g", [C], F32, kind="ExternalInput").ap()
    ln2b_d = nc.dram_tensor("ln2_b", [C], F32, kind="ExternalInput").ap()
    W1_d = nc.dram_tensor("W1", [C, FF], F32R, kind="ExternalInput").ap()
    b1_d = nc.dram_tensor("b1", [FF], F32, kind="ExternalInput").ap()
    W2_d = nc.dram_tensor("W2", [FF, C], F32R, kind="ExternalInput").ap()
    b2_d = nc.dram_tensor("b2", [C], F32, kind="ExternalInput").ap()
    out_d = nc.dram_tensor("out", [T, C], F32, kind="ExternalOutput").ap()

    Wqkv_r = Wqkv_d.rearrange("(j p) m -> p j m", p=P)     # [128, 8, 3072]
    Wproj_r = Wproj_d.rearrange("(j p) m -> p j m", p=P)   # [128, 8, 1024]
    W1_r = W1_d.rearrange("(j p) m -> p j m", p=P)         # [128, 8, 4096]
    W2_r = W2_d.rearrange("(j p) m -> p j m", p=P)         # [128, 32, 1024]

    with nc.allow_low_precision(reason="fp32r matmul inputs (fp32 accum)"), \
         tile.TileContext(nc) as tc, ExitStack() as ctx:
        const = ctx.enter_context(tc.tile_pool(name="const", bufs=1))
        xpool = ctx.enter_context(tc.tile_pool(name="xpool", bufs=8))
        hpool = ctx.enter_context(tc.tile_pool(name="hpool", bufs=8))
        spool = ctx.enter_context(tc.tile_pool(name="spool", bufs=2))
        wpool = ctx.enter_context(tc.tile_pool(name="wpool", bufs=2))
        ps_mm = ctx.enter_context(tc.tile_pool(name="ps_mm", bufs=4, space="PSUM"))
        ps_pv = ctx.enter_context(tc.tile_pool(name="ps_pv", bufs=2, space="PSUM"))
        ps_tr = ctx.enter_context(tc.tile_pool(name="ps_tr", bufs=2, space="PSUM"))

        ident = const.tile([P, P], F32)
        make_identity(nc, ident[:])
        ident_r = const.tile([P, P], F32R)
        nc.scalar.activation(ident_r[:], ident[:], AF.Copy)
        ones_f = const.tile([P, 1], F32)
        nc.vector.memset(ones_f[:], 1.0)
        ones_col = const.tile([P, 1], F32R)
        nc.scalar.activation(ones_col[:], ones_f[:], AF.Copy)
        ones_rowf = const.tile([1, P], F32)
        nc.vector.memset(ones_rowf[:], 1.0)
        ones_row = const.tile([1, P], F32R)
        nc.scalar.activation(ones_row[:], ones_rowf[:], AF.Copy)
        eps_t = const.tile([1, 1], F32)
        nc.vector.memset(eps_t[:], EPS)
        zero_col = const.tile([P, 1], F32)
        nc.vector.memset(zero_col[:], 0.0)

        # causal masks for diagonal-band blocks: mask_d[r,c] = 1 if c-r >= d*128
        masks = []
        with tc.tile_pool(name="mbuild", bufs=4) as mbp:
            for di in range(4):
                mf = mbp.tile([P, 512], F32, tag="mf", name=f"mf{di}")
                nc.gpsimd.memset(mf[:], 1.0)
                nc.gpsimd.affine_select(
                    out=mf[:], in_=mf[:], pattern=[[1, 512]],
                    base=-di * P, channel_multiplier=-1,
                    compare_op=mybir.AluOpType.is_ge, fill=0.0)
                mk = const.tile([P, 512], F32R, tag=f"mask{di}", name=f"mask{di}")
                nc.scalar.activation(mk[:], mf[:], AF.Copy)
                masks.append(mk)

        # bias/param columns: col m = vec[m*128:(m+1)*128]
        bqkv_t = const.tile([P, 3 * NCH], F32)
        nc.sync.dma_start(bqkv_t[:], bqkv_d.rearrange("(m p) -> p m", p=P))
        bproj_t = const.tile([P, NCH], F32)
        nc.sync.dma_start(bproj_t[:], bproj_d.rearrange("(m p) -> p m", p=P))
        b1_t = const.tile([P, FF // P], F32)
        nc.sync.dma_start(b1_t[:], b1_d.rearrange("(m p) -> p m", p=P))
        b2_t = const.tile([P, NCH], F32)
        nc.sync.dma_start(b2_t[:], b2_d.rearrange("(m p) -> p m", p=P))
        ln1g_t = const.tile([P, NCH], F32)
        nc.sync.dma_start(ln1g_t[:], ln1g_d.rearrange("(m p) -> p m", p=P))
        ln1b_t = const.tile([P, NCH], F32)
        nc.sync.dma_start(ln1b_t[:], ln1b_d.rearrange("(m p) -> p m", p=P))
        ln2g_t = const.tile([P, NCH], F32)
        nc.sync.dma_start(ln2g_t[:], ln2g_d.rearrange("(m p) -> p m", p=P))
        ln2b_t = const.tile([P, NCH], F32)
        nc.sync.dma_start(ln2b_t[:], ln2b_d.rearrange("(m p) -> p m", p=P))

        # persistent feature-major x tiles; become r1 then out in place
        x_t = [xpool.tile([P, T], F32R, tag="x", name=f"x_fm{m}") for m in range(NCH)]

        def layernorm_fm(src, g_t, b_t, out_tag, out_name):
            """src: 8 [128,1024] FM f32r tiles -> 8 normalized FM f32r tiles."""
            sum_ps = [ps_mm.tile([1, 512], F32, tag="ps", name=f"{out_name}_sum{t}")
                      for t in range(NQ)]
            sq_ps = [ps_mm.tile([1, 512], F32, tag="ps", name=f"{out_name}_sq{t}")
                     for t in range(NQ)]
            for c in range(NCH):
                for t in range(NQ):
                    sq = spool.tile([P, 512], F32R, tag="sq",
                                    name=f"{out_name}_sqv{c}_{t}")
                    nc.vector.tensor_mul(sq[:], src[c][:, ts(t, 512)],
                                         src[c][:, ts(t, 512)])
                    nc.tensor.matmul(
                        sum_ps[t][:], ones_col[:], src[c][:, ts(t, 512)],
                        start=(c == 0), stop=(c == NCH - 1))
                    nc.tensor.matmul(
                        sq_ps[t][:], ones_col[:], sq[:],
                        start=(c == 0), stop=(c == NCH - 1))
            inv_t = spool.tile([1, T], F32R, tag="lnstat", name=f"{out_name}_inv")
            c0_t = spool.tile([1, T], F32R, tag="lnstat", name=f"{out_name}_c0")
            for t in range(NQ):
                mu = spool.tile([1, 512], F32R, tag="sm512", bufs=5,
                                name=f"{out_name}_mu{t}")
                var = spool.tile([1, 512], F32, tag="sm512", bufs=5,
                                 name=f"{out_name}_var{t}")
                nc.scalar.mul(mu[:], sum_ps[t][:], 1.0 / C)
                nc.scalar.mul(var[:], sq_ps[t][:], 1.0 / C)
                musq = spool.tile([1, 512], F32, tag="sm512", bufs=5,
                                  name=f"{out_name}_musq{t}")
                nc.vector.tensor_mul(musq[:], mu[:], mu[:])
                nc.vector.tensor_sub(var[:], var[:], musq[:])
                sd = spool.tile([1, 512], F32, tag="sm512", bufs=5,
                                name=f"{out_name}_sd{t}")
                nc.scalar.activation(sd[:], var[:], AF.Sqrt, bias=eps_t[:])
                nc.vector.reciprocal(inv_t[:, ts(t, 512)], sd[:])
                nc.vector.tensor_mul(c0_t[:, ts(t, 512)], mu[:], inv_t[:, ts(t, 512)])
                nc.scalar.mul(c0_t[:, ts(t, 512)], c0_t[:, ts(t, 512)], -1.0)
            invb = spool.tile([P, T], F32R, tag="lnbc", name=f"{out_name}_invb")
            c0b = spool.tile([P, T], F32R, tag="lnbc", name=f"{out_name}_c0b")
            for t in range(NQ):
                for row, dst in ((inv_t, invb), (c0_t, c0b)):
                    bps = ps_mm.tile([P, 512], F32, tag="ps",
                                     name=f"{out_name}_bc{t}")
                    nc.tensor.matmul(bps[:], ones_row[:],
                                     row[:, ts(t, 512)], start=True, stop=True)
                    nc.scalar.activation(dst[:, ts(t, 512)], bps[:], AF.Copy)
            outs = []
            for c in range(NCH):
                h = hpool.tile([P, T], F32R, tag=out_tag, name=f"{out_name}{c}")
                nc.vector.tensor_mul(h[:], src[c][:], invb[:])
                nc.vector.tensor_add(h[:], h[:], c0b[:])
                nc.scalar.activation(h[:], h[:], AF.Identity,
                                     bias=b_t[:, c:c + 1], scale=g_t[:, c:c + 1])
                outs.append(h)
            return outs

        def linear_mtile(dst, w_src3, m, src_tiles, bias_col, func, nk=NCH,
                         wtag="w", name="lin"):
            """dst[:, :] (+bias, func) = W[:, m-chunk]^T @ src ; contraction nk*128."""
            wt = wpool.tile([P, nk, P], F32R, tag=wtag, bufs=3, name=f"{name}_w{m}")
            nc.sync.dma_start(wt[:], w_src3[:, :, ts(m, P)])
            for t in range(NQ):
                ps = ps_mm.tile([P, 512], F32, tag="ps", name=f"{name}_ps{m}_{t}")
                for j in range(nk):
                    nc.tensor.matmul(ps[:], wt[:, j, :],
                                     src_tiles[j][:, ts(t, 512)],
                                     start=(j == 0), stop=(j == nk - 1))
                nc.scalar.activation(dst[:, ts(t, 512)], ps[:], func,
                                     bias=bias_col, scale=1.0)

        # ---------------- load x (token-major) and transpose to FM ----------
        with tc.tile_pool(name="qkvt", bufs=6) as qkvt, \
             tc.tile_pool(name="vaug", bufs=10) as vaugp, \
             tc.tile_pool(name="ptp", bufs=4) as ptp, \
             tc.tile_pool(name="ypool", bufs=8) as ypool:

            xtm = [qkvt.tile([P, C], F32, tag="qkv", name=f"xtm{i}") for i in range(NT)]
            for i in range(NT):
                nc.sync.dma_start(xtm[i][:], x_d[ts(i, P), :])
            for i in range(NT):
                for m in range(NCH):
                    pst = ps_tr.tile([P, P], F32, tag="tr", name=f"xtr{i}_{m}")
                    nc.tensor.transpose(pst[:], xtm[i][:, ts(m, P)], ident[:])
                    nc.scalar.activation(x_t[m][:, ts(i, P)], pst[:], AF.Copy)

            # ---------------- LN1 ----------------
            h1 = layernorm_fm(x_t, ln1g_t, ln1b_t, "h", "h1")

            # ---------------- per-head-block QKV + attention ----------------
            y_t = [ypool.tile([P, T], F32R, tag="y", name=f"y{hb}")
                   for hb in range(NCH)]
            for hb in range(NCH):
                q_t = qkvt.tile([P, T], F32R, tag="qkv", name=f"q{hb}")
                k_t = qkvt.tile([P, T], F32R, tag="qkv", name=f"k{hb}")
                v_t = qkvt.tile([P, T], F32, tag="qkv", name=f"v{hb}")
                linear_mtile(q_t[:], Wqkv_r, hb, h1, bqkv_t[:, hb:hb + 1],
                             AF.Identity, name="q")
                linear_mtile(k_t[:], Wqkv_r, NCH + hb, h1,
                             bqkv_t[:, NCH + hb:NCH + hb + 1], AF.Identity, name="k")
                linear_mtile(v_t[:], Wqkv_r, 2 * NCH + hb, h1,
                             bqkv_t[:, 2 * NCH + hb:2 * NCH + hb + 1], AF.Identity,
                             name="v")
                # v -> token-major, per-head layout with a ones column:
                # vaug[ki] = [128(Tk), 130] : cols 0..63 head A, 64 ones,
                #                            65..128 head B, 129 ones
                vaug = [vaugp.tile([P, 130], F32R, tag="vaug", name=f"va{hb}_{ki}")
                        for ki in range(NT)]
                for ki in range(NT):
                    pst = ps_tr.tile([P, P], F32, tag="tr", name=f"vtr{hb}_{ki}")
                    nc.tensor.transpose(pst[:], v_t[:, ts(ki, P)], ident[:])
                    dst = vaug[ki][:].rearrange("p (h c) -> p h c", h=2)[:, :, 0:64]
                    src = pst[:].rearrange("p (h c) -> p h c", h=2)
                    nc.scalar.activation(dst, src, AF.Copy)
                    nc.scalar.activation(vaug[ki][:, 64:65], ones_f[:], AF.Copy)
                    nc.scalar.activation(vaug[ki][:, 129:130], ones_f[:], AF.Copy)
                for qi in range(NQ):
                    kmax = 4 * qi + 3
                    pv = {p_: ps_pv.tile([P, 512], F32, tag="pv",
                                         name=f"pv{hb}_{p_}_{qi}")
                          for p_ in range(2)}
                    for ki in range(kmax + 1):
                        pts = {}
                        for p_ in range(2):
                            st = ps_mm.tile([P, 512], F32, tag="ps",
                                            name=f"st{hb}_{p_}_{qi}_{ki}")
                            nc.tensor.matmul(
                                st[:],
                                k_t[p_ * 64:(p_ + 1) * 64, ts(ki, P)],
                                q_t[p_ * 64:(p_ + 1) * 64, ts(qi, 512)],
                                start=True, stop=True)
                            pt = ptp.tile([P, 512], F32R, tag="pt",
                                          name=f"pt{hb}_{p_}_{qi}_{ki}")
                            nc.scalar.activation(pt[:], st[:], AF.Exp,
                                                 bias=zero_col[:], scale=SCALE)
                            if ki >= 4 * qi:  # diagonal-band block: DVE mask
                                ptm = ptp.tile([P, 512], F32R, tag="pt",
                                               name=f"ptm{hb}_{p_}_{qi}_{ki}")
                                nc.vector.tensor_mul(ptm[:], pt[:],
                                                     masks[ki - 4 * qi][:])
                                pt = ptm
                            pts[p_] = pt
                        for p_ in range(2):
                            nc.tensor.matmul(
                                pv[p_][0:65, :],
                                vaug[ki][:, p_ * 65:(p_ + 1) * 65],
                                pts[p_][:],
                                start=(ki == 0), stop=(ki == kmax))
                    for p_ in range(2):
                        dnrow = spool.tile([1, 512], F32, tag="sm512",
                                           bufs=5, name=f"dr{hb}_{p_}_{qi}")
                        nc.scalar.activation(dnrow[:], pv[p_][64:65, :], AF.Copy)
                        dn = spool.tile([1, 512], F32, tag="sm512",
                                        bufs=5, name=f"dn{hb}_{p_}_{qi}")
                        nc.vector.reciprocal_approx_fast(dn[:], dnrow[:])
                        dnr = spool.tile([1, 512], F32R, tag="sm512",
                                         bufs=5, name=f"dq{hb}_{p_}_{qi}")
                        nc.scalar.activation(dnr[:], dn[:], AF.Copy)
                        dnb = spool.tile([64, 512], F32R, tag="dnb",
                                         bufs=2, name=f"dnb{hb}_{p_}_{qi}")
                        bps = ps_mm.tile([P, 512], F32, tag="ps",
                                         name=f"dnbc{hb}_{p_}_{qi}")
                        nc.tensor.matmul(bps[0:64, :], ones_row[:, 0:64],
                                         dnr[:], start=True, stop=True)
                        nc.scalar.activation(dnb[:], bps[0:64, :], AF.Copy)
                        nc.vector.tensor_mul(
                            y_t[hb][p_ * 64:(p_ + 1) * 64, ts(qi, 512)],
                            pv[p_][0:64, :], dnb[:])

            # ---------------- proj + residual (into x_t in place) -----------
            for m in range(NCH):
                nc.scalar.activation(x_t[m][:], x_t[m][:], AF.Identity,
                                     bias=bproj_t[:, m:m + 1], scale=1.0)
                wt = wpool.tile([P, NCH, P], F32R, tag="w", bufs=3,
                                name=f"proj_w{m}")
                nc.sync.dma_start(wt[:], Wproj_r[:, :, ts(m, P)])
                for t in range(NQ):
                    ps = ps_mm.tile([P, 512], F32, tag="ps", name=f"proj_ps{m}_{t}")
                    for j in range(NCH):
                        nc.tensor.matmul(ps[:], wt[:, j, :],
                                         y_t[j][:, ts(t, 512)],
                                         start=(j == 0), stop=(j == NCH - 1))
                    nc.vector.tensor_add(x_t[m][:, ts(t, 512)],
                                         x_t[m][:, ts(t, 512)], ps[:])

        # ---------------- LN2 ----------------
        h2 = layernorm_fm(x_t, ln2g_t, ln2b_t, "h", "h2")

        # ---------------- FFN (two d_ff halves) + residual ----------------
        with tc.tile_pool(name="a1pool", bufs=16) as a1pool:
            for m in range(NCH):  # pre-add b2 so FFN2 eviction is a plain add
                nc.scalar.activation(x_t[m][:], x_t[m][:], AF.Identity,
                                     bias=b2_t[:, m:m + 1], scale=1.0)
            for half in range(2):
                a1 = []
                for mm_ in range(16):
                    mg = half * 16 + mm_
                    a = a1pool.tile([P, T], F32R, tag="a1", name=f"a1_{mg}")
                    linear_mtile(a[:], W1_r, mg, h2, b1_t[:, mg:mg + 1],
                                 AF.Relu, name=f"ffn1_{mg}")
                    a1.append(a)
                for m in range(NCH):
                    w2t = wpool.tile([P, 16, P], F32R, tag="w2", name=f"w2_{half}_{m}")
                    nc.sync.dma_start(
                        w2t[:], W2_r[:, half * 16:(half + 1) * 16, ts(m, P)])
                    for t in range(NQ):
                        ps = ps_mm.tile([P, 512], F32, tag="ps",
                                        name=f"ffn2_ps{half}_{m}_{t}")
                        for j in range(16):
                            nc.tensor.matmul(ps[:], w2t[:, j, :],
                                             a1[j][:, ts(t, 512)],
                                             start=(j == 0), stop=(j == 15))
                        nc.vector.tensor_add(x_t[m][:, ts(t, 512)],
                                             x_t[m][:, ts(t, 512)], ps[:])

            # ---------------- transpose result back to token-major ----------
            for i in range(NT):
                ot = a1pool.tile([P, C], F32, tag="a1", name=f"ot{i}")
                for m in range(NCH):
                    pst = ps_tr.tile([P, P], F32R, tag="tr", name=f"otr{i}_{m}")
                    nc.tensor.transpose(pst[:], x_t[m][:, ts(i, P)], ident_r[:])
                    nc.scalar.activation(ot[:, ts(m, P)], pst[:], AF.Copy)
                nc.sync.dma_start(out_d[ts(i, P), :], ot[:])

    nc.compile()
    return nc


_NC_CACHE = {}


def _get_nc():
    if "nc" not in _NC_CACHE:
        _NC_CACHE["nc"] = _build()
    return _NC_CACHE["nc"]


def kernel(**inputs):
    from concourse.bass_utils import run_bass_kernel_spmd

    nc = _get_nc()
    names = ["Wqkv", "bqkv", "Wproj", "bproj", "ln1_g", "ln1_b", "ln2_g",
             "ln2_b", "W1", "b1", "W2", "b2"]
    shared = {k: np.ascontiguousarray(np.asarray(inputs[k], dtype=np.float32))
              for k in names}
    x = np.asarray(inputs["x"], dtype=np.float32)
    in_maps = [dict(shared, x=np.ascontiguousarray(x[i])) for i in range(B)]
    res = run_bass_kernel_spmd(nc, in_maps, core_ids=list(range(B)))
    out = np.stack([res.results[i]["out"] for i in range(B)], axis=0)
    return out.astype(np.float32)
